# revision 25
# baseline (speedup 1.0000x reference)
"""Trainium2 Bass kernel for nn_AttentionModule (B=8, C=128, H=W=256), v3.

out[b,c] = softmax((W1 x_b + b1)[c] @ ((W2 x_b + b2)[c])^T) @ ((W2 x_b + b2)[c]) + x_b[c]

Sharding: data-parallel over batch B across the 8 NeuronCores (1 batch each);
weights replicated. Each core runs an identical single-core NEFF.

v3 redesign vs v2 (402.9us): compute scoresT = k q^T instead of scores, so
softmax rows live on PSUM partitions and a FIXED exp shift replaces the
per-row max (any per-row factor cancels in the normalization):
  - kills the DVE max-reduce, the P transpose (PE), and the attnT evac (DVE)
  - P := exp(scoresT - 75) is stored in bf16 (fp32-like range, so the fixed
    shift cannot overflow; 8-bit mantissa adds ~3e-3 error vs the 2e-2 gate)
  - out-mm: lhsT = P^T (bf16) x rhs = [knat | ones] (fp16); the ones column
    accumulates the softmax row-sum l_h for free (as v2)
Phase A: single input stream xres = x + delta with delta = (I+W2)^-1 b2:
  makes k = W2 xres exactly the unbiased k whose per-channel shift folds
  into the residual (which is xres again), so the k evac needs NO bias and
  the whole 512-elem PSUM bank evacuates with ONE tensor_copy (alternating
  DVE/ACT). q and k share one h-contiguous layout [wc][t][c][h]. The q bias
  (b1 - W1 delta) is applied afterwards by one 2x-mode fp16 SBUF tensor_add
  per (group, wc) against a small broadcast-pattern bias tile.

Container workarounds (see _apply_tile_patches):
  - walrus here encodes at most one sem wait per instruction -> split.
  - EVSEM butterfly barrier hangs at runtime -> NRT pseudo barrier.
  - sem_clear/dma_reset hang -> skipped (one execution per model load).
  - HWDGE (nc.sync) DMAs hang under Tile -> all DMAs on gpsimd (SWDGE).
"""

import sys

if '/opt/trn_rl_repo' not in sys.path:
    sys.path.insert(0, '/opt/trn_rl_repo')

import numpy as np

B, C, H, W = 8, 128, 256, 256
G = 64            # channels per group
NG = C // G       # 2 groups / x passes
N_CORES = 8
HW_ELEMS = H * W
SHIFT = 75.0      # fixed exp shift; scores observed in [-143, 153]

# Tunables: engine rotation per op class ('v' = DVE, 's' = ACT) and DMA
# batch sizes.
CFG = dict(
    evac="vs",          # Phase A combined bank evac rotation
    knat="v",           # Phase B knat evac
    # final out = po*rinv + xres: 'v' = DVE scalar_tensor_tensor;
    # 'a' = ACT scale-copy to fp16 tmp + GPSIMD sbuf-only add.
    final="avv",
    chbatch=8,          # channels per xres/out DMA
    xrows=32,           # h rows per streamed x chunk
    biasch=8,           # channels per q-bias op (split to overlap evac)
    lookahead=2,        # Phase B software pipeline depth
)

HCHUNK = CFG['xrows']
QKH = G * H       # 16384: cols per (wc, t) block of the qk tile

_patched = False


def _apply_tile_patches():
    global _patched
    if _patched:
        return
    _patched = True
    import concourse.tile as tile
    from concourse.vector_clock import ScopedClock

    def _drain_and_barrier(self, tick_clock, wait_clock):
        nc = self.nc
        drain_inst = nc.sync.drain()
        wait_clock.add_sem_waits(
            drain_inst.ins, ScopedClock({None: tick_clock.global_clock})
        )
        nc._nrt_pseudo_barrier()
        assert self.sems is not None
        popped = nc._tile_sem_poison_stack.pop()
        assert popped is self._sem_poison
        # No sem_clear / dma_reset: RANGE_CLEAR and DMA_RESET hang on this
        # runtime. Sound because every kernel() call loads a fresh
        # executable (NRT zeroes semaphores at load).

    tile.TileContext._drain_and_barrier = _drain_and_barrier


def _split_multi_waits(nc):
    from concourse import mybir
    n = 0
    for f in nc.m.functions:
        for blk in f.blocks:
            insts = list(blk.instructions)
            out = []
            changed = False
            for inst in insts:
                si = getattr(inst, "sync_info", None)
                if si is not None and len(si.on_wait) > 1:
                    waits = list(si.on_wait)
                    for i, w in enumerate(waits[:-1]):
                        nop = mybir.InstNoOp(
                            name=f"{inst.name}_wsplit{i}", ins=[], outs=[])
                        nop.engine = inst.engine
                        nop.sync_info = mybir.SyncInfo(on_wait=[w], on_update=[])
                        out.append(nop)
                        n += 1
                    inst.sync_info = mybir.SyncInfo(
                        on_wait=[waits[-1]], on_update=list(si.on_update))
                    changed = True
                out.append(inst)
            if changed:
                blk.instructions = out
    return n


def build_program(patch=True):
    """Build the single-core Bass program. Returns nc."""
    if patch:
        _apply_tile_patches()
    import concourse.bass as bass
    import concourse.tile as tile
    from concourse import mybir
    from contextlib import ExitStack

    f32 = mybir.dt.float32
    f16 = mybir.dt.float16
    bf16 = mybir.dt.bfloat16
    AF = mybir.ActivationFunctionType
    ALU = mybir.AluOpType

    nc = bass.Bass("TRN2", target_bir_lowering=False, debug=False, num_devices=1)
    xres_t = nc.dram_tensor("xres", [C, H, W], f16, kind="ExternalInput")
    wcat_t = nc.dram_tensor("wcat", [128, 2 * C], f16, kind="ExternalInput")
    biasq_t = nc.dram_tensor("biasq", [128, 2 * C], f16, kind="ExternalInput")
    ident_t = nc.dram_tensor("ident", [128, 128], f16, kind="ExternalInput")
    out_t = nc.dram_tensor("out", [C, H, W], f16, kind="ExternalOutput")

    def dram_ch(tensor, c, nch):
        # [h(128 part), ch(nch), ht(2), w] view of nch channels from [C,H,W]
        return bass.AP(tensor.ap().tensor, c * HW_ELEMS,
                       [[W, 128], [HW_ELEMS, nch], [128 * W, 2], [1, W]])

    def sb4(t, d1, d2, d3):
        # 4D view [part, d1, d2, d3] of an sbuf tile AP (flat col-major)
        return bass.AP(t.tensor, t.offset, [t.ap[0], d1, d2, d3])

    with tile.TileContext(nc) as tc, ExitStack() as ctx:
        consts = ctx.enter_context(tc.tile_pool(name="consts", bufs=1))
        gqk = ctx.enter_context(tc.tile_pool(name="gqk", bufs=1))
        xpool = ctx.enter_context(tc.tile_pool(name="xpool", bufs=2))
        ppool = ctx.enter_context(tc.tile_pool(name="ppool", bufs=4))
        knpool = ctx.enter_context(tc.tile_pool(name="knpool", bufs=1))
        opool = ctx.enter_context(tc.tile_pool(name="opool", bufs=2))
        xrpool = ctx.enter_context(tc.tile_pool(name="xrpool", bufs=2))
        tmppool = ctx.enter_context(tc.tile_pool(name="tmppool", bufs=3))
        stats = ctx.enter_context(tc.tile_pool(name="stats", bufs=4))
        # PSUM: 8 banks. pb [128,512]f32 x3 (Phase A accumulators AND
        # Phase B scoresT); po [128,1024]f32 x2 (out-mm, 2 ht halves in one
        # 2-bank tile); pk [128,512]f16 x2 (knat transposes, half-bank each).
        psum = ctx.enter_context(tc.tile_pool(name="psum", bufs=3, space="PSUM"))

        # startup order matters: wcat first (gates the first matmul), then
        # the first two x chunks, then the cold-path consts.
        wcat_sb = consts.tile([128, 2 * C], f16)
        nc.gpsimd.dma_start(out=wcat_sb[:], in_=wcat_t.ap())
        def xt_fetch(hc):
            xt = xpool.tile([128, HCHUNK * W], f16, tag="xt", name="xt")
            nc.gpsimd.dma_start(
                out=xt[:].rearrange("p (a b) -> p a b", a=HCHUNK),
                in_=xres_t.ap()[:, hc * HCHUNK:(hc + 1) * HCHUNK, :])
            return xt

        xt_pre = xt_fetch(0)
        ident_sb = consts.tile([128, 128], f16)
        nc.gpsimd.dma_start(out=ident_sb[:], in_=ident_t.ap())
        biasq_sb = consts.tile([128, 2 * C], f16)
        nc.gpsimd.dma_start(out=biasq_sb[:], in_=biasq_t.ap())

        eng = {'v': nc.vector, 's': nc.scalar}

        shift_sb = consts.tile([128, 1], f32)
        nc.vector.memset(shift_sb[:], -SHIFT)

        # knat tiles carry preset ones columns at cols 256 and 513; evacs
        # only write the w columns, so the ones persist.
        kn_ring = []
        for i in range(4):
            kn = knpool.tile([128, 514], f16, tag=f"knat{i}", bufs=1,
                             name=f"knr{i}")
            ones_ap = bass.AP(kn[:].tensor, kn[:].offset + 256,
                              [kn[:].ap[0], [257, 2]])
            nc.vector.memset(ones_ap, 1.0)
            kn_ring.append(kn)

        # Phase A PSUM rotation covering all 8 banks: 3x single-bank (pb)
        # + 2x double-bank (po, 8 mms each) + 1x single (pk slot, f32).
        AROT = [("pb", 4, 3), ("pb", 4, 3), ("pb", 4, 3),
                ("po", 8, 2), ("po", 8, 2), ("pk", 4, 1)]
        assert sum(nh for _, nh, _ in AROT) == HCHUNK

        bank_i = 0
        for g in range(NG):
            # group-resident q/k: [128(w), wc(2) x t(2) x c(64) x h(256)]
            # fp16, h innermost (contiguous stage1 rhs / lhsT slices).
            qk = gqk.tile([128, 2 * 2 * G * H], f16, tag="qk")

            # prefetch the first Phase B xres batch during Phase A
            xr0 = xrpool.tile([128, CFG['chbatch'] * 512], f16, tag="xr")
            nc.gpsimd.dma_start(
                out=sb4(xr0[:], [512, CFG['chbatch']], [256, 2], [1, 256]),
                in_=dram_ch(xres_t, g * G, CFG['chbatch']))

            # ---------------- Phase A ----------------
            for hc in range(H // HCHUNK):
                if xt_pre is not None and hc == 0:
                    xt = xt_pre
                    xt_pre = None
                else:
                    xt = xt_fetch(hc)
                for wc in range(2):
                    hb = 0
                    for tag, nh, nbufs in AROT:
                        ps = psum.tile([128, nh * 128], f32, tag=tag,
                                       bufs=nbufs, name="pa")
                        for i in range(nh):
                            nc.tensor.matmul(
                                out=ps[:, i * 128:(i + 1) * 128],
                                lhsT=xt[:, (hb + i) * W + wc * 128:
                                        (hb + i) * W + wc * 128 + 128],
                                rhs=wcat_sb[:, g * 128:(g + 1) * 128],
                                start=True, stop=True)
                        # one combined evac per tile: value (t, c, i) sits at
                        # PSUM col i*128 + t*64 + c and lands at qk col
                        # wc*2*QKH + t*QKH + c*H + (h0 + i)
                        h0 = hc * HCHUNK + hb
                        ps_in = bass.AP(ps[:].tensor, ps[:].offset,
                                        [ps[:].ap[0], [64, 2], [1, G], [128, nh]])
                        qk_out = bass.AP(qk[:].tensor,
                                         qk[:].offset + wc * 2 * QKH + h0,
                                         [qk[:].ap[0], [QKH, 2], [H, G], [1, nh]])
                        ee = CFG['evac'][bank_i % len(CFG['evac'])]
                        if ee == 's':
                            nc.scalar.activation(qk_out, ps_in, AF.Copy)
                        else:
                            nc.vector.tensor_copy(qk_out, ps_in)
                        bank_i += 1
                        hb += nh
                    # q bias for this (hc, wc) h-range: one 2x-mode fp16 add
                    # (overlaps later chunks' evacs). biasq col layout:
                    # g*128 + c*2 + j, value alpha_q[g*G+c]; broadcast over h
                    # via a stride-0 middle dim.
                    q_ap = bass.AP(
                        qk[:].tensor,
                        qk[:].offset + wc * 2 * QKH + hc * HCHUNK,
                        [qk[:].ap[0], [H, G], [2, HCHUNK // 2], [1, 2]])
                    b_ap = bass.AP(
                        biasq_sb[:].tensor, biasq_sb[:].offset + g * 128,
                        [biasq_sb[:].ap[0], [2, G], [0, HCHUNK // 2], [1, 2]])
                    nc.vector.tensor_add(q_ap, q_ap, b_ap)

            # ---------------- Phase B ----------------
            def q_sl(cl, wc):
                # qT for channel cl, w-chunk wc: [128 w, 256 h] contiguous
                o = wc * 2 * QKH + cl * H
                return qk[:, o:o + H]

            def k_sl(cl, wc, n=H, o2=0):
                o = wc * 2 * QKH + QKH + cl * H + o2
                return qk[:, o:o + n]

            def stage1(cl):
                # scoresT block gc: [g(128 part), h(256)] at sc cols gc*256
                sc = psum.tile([128, 512], f32, tag="pb", bufs=3)
                for gc in range(2):
                    nc.tensor.matmul(out=sc[:, gc * 256:gc * 256 + 256],
                                     lhsT=k_sl(cl, 0, 128, gc * 128),
                                     rhs=q_sl(cl, 0),
                                     start=True, stop=False)
                    nc.tensor.matmul(out=sc[:, gc * 256:gc * 256 + 256],
                                     lhsT=k_sl(cl, 1, 128, gc * 128),
                                     rhs=q_sl(cl, 1),
                                     start=False, stop=True)
                P = ppool.tile([128, 512], bf16, tag="P")
                nc.scalar.activation(P[:], sc[:], AF.Exp, bias=shift_sb[:],
                                     scale=1.0)
                return P

            def transp(cl):
                # kT quadrants -> knat [g part, w cols] (+ preset ones)
                pkt = psum.tile([128, 512], f16, tag="pk", bufs=1)
                for gc in range(2):
                    for wc in range(2):
                        nc.tensor.matmul(
                            out=pkt[:, gc * 256 + wc * 128:gc * 256 + wc * 128 + 128],
                            lhsT=k_sl(cl, wc, 128, gc * 128),
                            rhs=ident_sb[:], is_transpose=True,
                            start=(wc == 0), stop=(wc == 1))
                kn = kn_ring[(g * G + cl) % 4]
                kn_out = bass.AP(kn[:].tensor, kn[:].offset,
                                 [kn[:].ap[0], [257, 2], [1, 256]])
                pk_in = bass.AP(pkt[:].tensor, pkt[:].offset,
                                [pkt[:].ap[0], [256, 2], [1, 256]])
                ke = CFG['knat'][cl % len(CFG['knat'])]
                if ke == 's':
                    nc.scalar.activation(kn_out, pk_in, AF.Copy)
                else:
                    nc.vector.tensor_copy(kn_out, pk_in)
                return kn

            def stage2(cl, P, kn, ob, xr, obslot):
                # out[h,w] (+ l in col 256) = sum_gc P^T block @ [knat|ones]
                po = psum.tile([128, 1024], f32, tag="po", bufs=2)
                for ht in range(2):
                    for gc in range(2):
                        nc.tensor.matmul(
                            out=po[:, ht * 512:ht * 512 + 257],
                            lhsT=P[:, gc * 256 + ht * 128:gc * 256 + ht * 128 + 128],
                            rhs=kn[:, gc * 257:gc * 257 + 257],
                            start=(gc == 0), stop=(gc == 1))
                rinv = stats.tile([128, 2], f32, tag="rinv")
                l_ap = bass.AP(po[:].tensor, po[:].offset + 256,
                               [po[:].ap[0], [512, 2]])
                nc.vector.reciprocal(rinv[:], l_ap)
                fe = CFG['final'][cl % len(CFG['final'])]
                if fe == 'v':
                    for ht in range(2):
                        nc.vector.scalar_tensor_tensor(
                            out=ob[:, obslot * 512 + ht * 256:obslot * 512 + ht * 256 + 256],
                            in0=po[:, ht * 512:ht * 512 + 256],
                            scalar=rinv[:, ht:ht + 1],
                            in1=xr[:, obslot * 512 + ht * 256:obslot * 512 + ht * 256 + 256],
                            op0=ALU.mult, op1=ALU.add)
                else:
                    tmp = tmppool.tile([128, 512], f16, tag="tmp")
                    for ht in range(2):
                        nc.scalar.activation(
                            tmp[:, ht * 256:ht * 256 + 256],
                            po[:, ht * 512:ht * 512 + 256],
                            AF.Copy, scale=rinv[:, ht:ht + 1])
                    nc.gpsimd.tensor_add(
                        ob[:, obslot * 512:obslot * 512 + 512], tmp[:],
                        xr[:, obslot * 512:obslot * 512 + 512])

            # CB-channel batches for xres-in and out DMAs; xres prefetched
            # one batch ahead. Software pipeline with D channels lookahead:
            # stage2(cl-D) runs after stage1/transp(cl), giving exp/knat a
            # full extra channel of slack before the out-mm consumes them.
            CB = CFG['chbatch']
            D = CFG['lookahead']

            def xr_fetch(b):
                xr = xrpool.tile([128, CB * 512], f16, tag="xr")
                nc.gpsimd.dma_start(
                    out=sb4(xr[:], [512, CB], [256, 2], [1, 256]),
                    in_=dram_ch(xres_t, g * G + b * CB, CB))
                return xr

            xrq = [xr0]
            pend = []
            obs = {}
            for cl in range(G + D):
                if cl == G - 16 and g + 1 < NG:
                    # prefetch next group's first x chunk during Phase B
                    xt_pre = xt_fetch(0)
                if cl < G:
                    if cl % CB == 0:
                        obs[cl // CB] = opool.tile([128, CB * 512], f16,
                                                   tag="ob", name="ob")
                        if cl // CB + 1 < G // CB:
                            xrq.append(xr_fetch(cl // CB + 1))
                    P = stage1(cl)
                    kn = transp(cl)
                    pend.append((cl, P, kn))
                if cl >= D:
                    c0, P0, kn0 = pend.pop(0)
                    b = c0 // CB
                    stage2(c0, P0, kn0, obs[b], xrq[0], c0 % CB)
                    if c0 % CB == CB - 1:
                        nc.gpsimd.dma_start(
                            out=dram_ch(out_t, g * G + b * CB, CB),
                            in_=sb4(obs[b][:], [512, CB], [256, 2], [1, 256]))
                        xrq.pop(0)
                        del obs[b]
    return nc


def _host_consts(W1, b1, W2, b2):
    delta = np.linalg.solve(
        np.eye(C, dtype=np.float64) + np.asarray(W2, np.float64),
        np.asarray(b2, np.float64))
    alpha_q = np.asarray(b1, np.float64) - np.asarray(W1, np.float64) @ delta
    wcat = np.empty((128, 2 * C), np.float16)
    for g in range(NG):
        for t, Wm in ((0, W1), (1, W2)):
            for cl in range(G):
                wcat[:, g * 128 + t * G + cl] = np.float16(Wm[g * G + cl, :])
    biasq = np.zeros((128, 2 * C), np.float16)
    for g in range(NG):
        for cl in range(G):
            biasq[:, g * 128 + cl * 2:g * 128 + cl * 2 + 2] = np.float16(
                alpha_q[g * G + cl])
    ident = np.eye(128, dtype=np.float16)
    return delta, {"wcat": wcat, "biasq": biasq, "ident": ident}


def _host_inputs(x_b, delta, consts):
    xres = (np.asarray(x_b, np.float64)
            + delta[:, None, None]).astype(np.float16)
    return {"xres": xres, **consts}


def kernel(x, W1, b1, W2, b2, _trace=False):
    import concourse.bass_utils as bass_utils

    nc = build_program(patch=True)
    _split_multi_waits(nc)

    delta, consts = _host_consts(W1, b1, W2, b2)
    in_maps = [_host_inputs(x[b], delta, consts) for b in range(B)]
    kw = {}
    if _trace:
        kw = dict(trace=True, trace_cores=[0])
    res = bass_utils.run_bass_kernel_spmd(
        nc, in_maps, core_ids=list(range(N_CORES)), **kw)
    out = np.stack([res.results[b]["out"] for b in range(B)], axis=0)
    if _trace:
        kernel._last_results = res
    return out.astype(np.float32)


# revision 32
# speedup vs baseline: 1.0004x; 1.0004x over previous
"""Trainium2 Bass kernel for nn_AttentionModule (B=8, C=128, H=W=256), v3.

out[b,c] = softmax((W1 x_b + b1)[c] @ ((W2 x_b + b2)[c])^T) @ ((W2 x_b + b2)[c]) + x_b[c]

Sharding: data-parallel over batch B across the 8 NeuronCores (1 batch each);
weights replicated. Each core runs an identical single-core NEFF.

v3 redesign vs v2 (402.9us): compute scoresT = k q^T instead of scores, so
softmax rows live on PSUM partitions and a FIXED exp shift replaces the
per-row max (any per-row factor cancels in the normalization):
  - kills the DVE max-reduce, the P transpose (PE), and the attnT evac (DVE)
  - P := exp(scoresT - 75) is stored in bf16 (fp32-like range, so the fixed
    shift cannot overflow; 8-bit mantissa adds ~3e-3 error vs the 2e-2 gate)
  - out-mm: lhsT = P^T (bf16) x rhs = [knat | ones] (fp16); the ones column
    accumulates the softmax row-sum l_h for free (as v2)
Phase A: single input stream xres = x + delta with delta = (I+W2)^-1 b2:
  makes k = W2 xres exactly the unbiased k whose per-channel shift folds
  into the residual (which is xres again), so the k evac needs NO bias and
  the whole 512-elem PSUM bank evacuates with ONE tensor_copy (alternating
  DVE/ACT). q and k share one h-contiguous layout [wc][t][c][h]. The q bias
  (b1 - W1 delta) is applied afterwards by one 2x-mode fp16 SBUF tensor_add
  per (group, wc) against a small broadcast-pattern bias tile.

Container workarounds (see _apply_tile_patches):
  - walrus here encodes at most one sem wait per instruction -> split.
  - EVSEM butterfly barrier hangs at runtime -> NRT pseudo barrier.
  - sem_clear/dma_reset hang -> skipped (one execution per model load).
  - HWDGE (nc.sync) DMAs hang under Tile -> all DMAs on gpsimd (SWDGE).
"""

import sys

if '/opt/trn_rl_repo' not in sys.path:
    sys.path.insert(0, '/opt/trn_rl_repo')

import numpy as np

B, C, H, W = 8, 128, 256, 256
G = 64            # channels per group
NG = C // G       # 2 groups / x passes
N_CORES = 8
HW_ELEMS = H * W
SHIFT = 75.0      # fixed exp shift; scores observed in [-143, 153]

# Tunables: engine rotation per op class ('v' = DVE, 's' = ACT) and DMA
# batch sizes.
CFG = dict(
    evac="vs",          # Phase A combined bank evac rotation
    knat="v",           # Phase B knat evac
    # final out = po*rinv + xres: 'v' = DVE scalar_tensor_tensor;
    # 'a' = ACT scale-copy to fp16 tmp + GPSIMD sbuf-only add.
    final="avv",
    chbatch=4,          # channels per xres/out DMA
    xrows=32,           # h rows per streamed x chunk
    biasch=8,           # channels per q-bias op (split to overlap evac)
    lookahead=2,        # Phase B software pipeline depth
)

HCHUNK = CFG['xrows']
QKH = G * H       # 16384: cols per (wc, t) block of the qk tile

_patched = False


def _apply_tile_patches():
    global _patched
    if _patched:
        return
    _patched = True
    import concourse.tile as tile
    from concourse.vector_clock import ScopedClock

    def _drain_and_barrier(self, tick_clock, wait_clock):
        nc = self.nc
        drain_inst = nc.sync.drain()
        wait_clock.add_sem_waits(
            drain_inst.ins, ScopedClock({None: tick_clock.global_clock})
        )
        nc._nrt_pseudo_barrier()
        assert self.sems is not None
        popped = nc._tile_sem_poison_stack.pop()
        assert popped is self._sem_poison
        # No sem_clear / dma_reset: RANGE_CLEAR and DMA_RESET hang on this
        # runtime. Sound because every kernel() call loads a fresh
        # executable (NRT zeroes semaphores at load).

    tile.TileContext._drain_and_barrier = _drain_and_barrier


def _split_multi_waits(nc):
    from concourse import mybir
    n = 0
    for f in nc.m.functions:
        for blk in f.blocks:
            insts = list(blk.instructions)
            out = []
            changed = False
            for inst in insts:
                si = getattr(inst, "sync_info", None)
                if si is not None and len(si.on_wait) > 1:
                    waits = list(si.on_wait)
                    for i, w in enumerate(waits[:-1]):
                        nop = mybir.InstNoOp(
                            name=f"{inst.name}_wsplit{i}", ins=[], outs=[])
                        nop.engine = inst.engine
                        nop.sync_info = mybir.SyncInfo(on_wait=[w], on_update=[])
                        out.append(nop)
                        n += 1
                    inst.sync_info = mybir.SyncInfo(
                        on_wait=[waits[-1]], on_update=list(si.on_update))
                    changed = True
                out.append(inst)
            if changed:
                blk.instructions = out
    return n


def build_program(patch=True):
    """Build the single-core Bass program. Returns nc."""
    if patch:
        _apply_tile_patches()
    import concourse.bass as bass
    import concourse.tile as tile
    from concourse import mybir
    from contextlib import ExitStack

    f32 = mybir.dt.float32
    f16 = mybir.dt.float16
    bf16 = mybir.dt.bfloat16
    AF = mybir.ActivationFunctionType
    ALU = mybir.AluOpType

    nc = bass.Bass("TRN2", target_bir_lowering=False, debug=False, num_devices=1)
    xres_t = nc.dram_tensor("xres", [C, H, W], f16, kind="ExternalInput")
    wcat_t = nc.dram_tensor("wcat", [128, 2 * C], f16, kind="ExternalInput")
    biasq_t = nc.dram_tensor("biasq", [128, NG * 512], f16,
                             kind="ExternalInput")
    ident_t = nc.dram_tensor("ident", [128, 128], f16, kind="ExternalInput")
    out_t = nc.dram_tensor("out", [C, H, W], f16, kind="ExternalOutput")

    def dram_ch(tensor, c, nch):
        # [h(128 part), ch(nch), ht(2), w] view of nch channels from [C,H,W]
        return bass.AP(tensor.ap().tensor, c * HW_ELEMS,
                       [[W, 128], [HW_ELEMS, nch], [128 * W, 2], [1, W]])

    def sb4(t, d1, d2, d3):
        # 4D view [part, d1, d2, d3] of an sbuf tile AP (flat col-major)
        return bass.AP(t.tensor, t.offset, [t.ap[0], d1, d2, d3])

    with tile.TileContext(nc) as tc, ExitStack() as ctx:
        consts = ctx.enter_context(tc.tile_pool(name="consts", bufs=1))
        gqk = ctx.enter_context(tc.tile_pool(name="gqk", bufs=1))
        xpool = ctx.enter_context(tc.tile_pool(name="xpool", bufs=2))
        ppool = ctx.enter_context(tc.tile_pool(name="ppool", bufs=4))
        knpool = ctx.enter_context(tc.tile_pool(name="knpool", bufs=1))
        opool = ctx.enter_context(tc.tile_pool(name="opool", bufs=2))
        xrpool = ctx.enter_context(tc.tile_pool(name="xrpool", bufs=3))
        tmppool = ctx.enter_context(tc.tile_pool(name="tmppool", bufs=3))
        stats = ctx.enter_context(tc.tile_pool(name="stats", bufs=4))
        # PSUM: 8 banks. pb [128,512]f32 x3 (Phase A accumulators AND
        # Phase B scoresT); po [128,1024]f32 x2 (out-mm, 2 ht halves in one
        # 2-bank tile); pk [128,512]f16 x2 (knat transposes, half-bank each).
        psum = ctx.enter_context(tc.tile_pool(name="psum", bufs=3, space="PSUM"))

        # startup order matters: wcat first (gates the first matmul), then
        # the first two x chunks, then the cold-path consts.
        wcat_sb = consts.tile([128, 2 * C], f16)
        nc.gpsimd.dma_start(out=wcat_sb[:], in_=wcat_t.ap())
        def xt_fetch(hc):
            xt = xpool.tile([128, HCHUNK * W], f16, tag="xt", name="xt")
            nc.gpsimd.dma_start(
                out=xt[:].rearrange("p (a b) -> p a b", a=HCHUNK),
                in_=xres_t.ap()[:, hc * HCHUNK:(hc + 1) * HCHUNK, :])
            return xt

        xt_pre = xt_fetch(0)
        xt_pre2 = xt_fetch(1)
        biasq_sb = consts.tile([128, NG * 512], f16)
        nc.gpsimd.dma_start(out=biasq_sb[:], in_=biasq_t.ap())
        ident_sb = consts.tile([128, 128], f16)
        nc.gpsimd.dma_start(out=ident_sb[:], in_=ident_t.ap())

        eng = {'v': nc.vector, 's': nc.scalar}

        shift_sb = consts.tile([128, 1], f32)
        nc.vector.memset(shift_sb[:], -SHIFT)

        # knat tiles carry preset ones columns at cols 256 and 513; evacs
        # only write the w columns, so the ones persist.
        kn_ring = []
        for i in range(4):
            kn = knpool.tile([128, 514], f16, tag=f"knat{i}", bufs=1,
                             name=f"knr{i}")
            ones_ap = bass.AP(kn[:].tensor, kn[:].offset + 256,
                              [kn[:].ap[0], [257, 2]])
            nc.vector.memset(ones_ap, 1.0)
            kn_ring.append(kn)

        # Phase A PSUM rotation covering all 8 banks: 3x single-bank (pb)
        # + 2x double-bank (po, 8 mms each) + 1x single (pk slot, f32).
        AROT = [("pb", 4, 3), ("pb", 4, 3), ("pb", 4, 3),
                ("po", 8, 2), ("po", 8, 2), ("pk", 4, 1)]
        assert sum(nh for _, nh, _ in AROT) == HCHUNK

        bank_i = 0
        for g in range(NG):
            # group-resident q/k: [128(w), wc(2) x t(2) x c(64) x h(256)]
            # fp16, h innermost (contiguous stage1 rhs / lhsT slices).
            qk = gqk.tile([128, 2 * 2 * G * H], f16, tag="qk")

            # prefetch the first Phase B xres batch during Phase A
            xr0 = xrpool.tile([128, CFG['chbatch'] * 512], f16, tag="xr")
            nc.gpsimd.dma_start(
                out=sb4(xr0[:], [512, CFG['chbatch']], [256, 2], [1, 256]),
                in_=dram_ch(xres_t, g * G, CFG['chbatch']))

            # ---------------- Phase A ----------------
            for hc in range(H // HCHUNK):
                if xt_pre is not None and hc == 0:
                    xt = xt_pre
                    xt_pre = None
                elif xt_pre2 is not None and hc == 1:
                    xt = xt_pre2
                    xt_pre2 = None
                else:
                    xt = xt_fetch(hc)
                for wc in range(2):
                    hb = 0
                    for tag, nh, nbufs in AROT:
                        ps = psum.tile([128, nh * 128], f32, tag=tag,
                                       bufs=nbufs, name="pa")
                        for i in range(nh):
                            nc.tensor.matmul(
                                out=ps[:, i * 128:(i + 1) * 128],
                                lhsT=xt[:, (hb + i) * W + wc * 128:
                                        (hb + i) * W + wc * 128 + 128],
                                rhs=wcat_sb[:, g * 128:(g + 1) * 128],
                                start=True, stop=True)
                        # split evac per tile: DVE writes the q half via
                        # scalar_tensor_tensor with the q bias fused in
                        # (stt ≈ copy cost: fp32 PSUM reads get no DVE perf
                        # modes anyway); ACT copies the bias-free k half.
                        # PSUM col i*128 + t*64 + c -> qk col
                        # wc*2*QKH + t*QKH + c*H + (h0 + i)
                        h0 = hc * HCHUNK + hb
                        ps_q = bass.AP(ps[:].tensor, ps[:].offset,
                                       [ps[:].ap[0], [1, G], [128, nh]])
                        ps_k = bass.AP(ps[:].tensor, ps[:].offset + G,
                                       [ps[:].ap[0], [1, G], [128, nh]])
                        q_out = bass.AP(qk[:].tensor,
                                        qk[:].offset + wc * 2 * QKH + h0,
                                        [qk[:].ap[0], [H, G], [1, nh]])
                        k_out = bass.AP(qk[:].tensor,
                                        qk[:].offset + wc * 2 * QKH + QKH + h0,
                                        [qk[:].ap[0], [H, G], [1, nh]])
                        # bias tile col g*512 + c*8 + i == alpha_q[g*G+c]
                        b_ap = bass.AP(biasq_sb[:].tensor,
                                       biasq_sb[:].offset + g * 512,
                                       [biasq_sb[:].ap[0], [8, G], [1, nh]])
                        nc.vector.scalar_tensor_tensor(
                            out=q_out, in0=ps_q, scalar=1.0, in1=b_ap,
                            op0=ALU.mult, op1=ALU.add)
                        nc.scalar.activation(k_out, ps_k, AF.Copy)
                        bank_i += 1
                        hb += nh

            # ---------------- Phase B ----------------
            def q_sl(cl, wc):
                # qT for channel cl, w-chunk wc: [128 w, 256 h] contiguous
                o = wc * 2 * QKH + cl * H
                return qk[:, o:o + H]

            def k_sl(cl, wc, n=H, o2=0):
                o = wc * 2 * QKH + QKH + cl * H + o2
                return qk[:, o:o + n]

            def stage1(cl):
                # scoresT block gc: [g(128 part), h(256)] at sc cols gc*256
                sc = psum.tile([128, 512], f32, tag="pb", bufs=3)
                for gc in range(2):
                    nc.tensor.matmul(out=sc[:, gc * 256:gc * 256 + 256],
                                     lhsT=k_sl(cl, 0, 128, gc * 128),
                                     rhs=q_sl(cl, 0),
                                     start=True, stop=False)
                    nc.tensor.matmul(out=sc[:, gc * 256:gc * 256 + 256],
                                     lhsT=k_sl(cl, 1, 128, gc * 128),
                                     rhs=q_sl(cl, 1),
                                     start=False, stop=True)
                P = ppool.tile([128, 512], bf16, tag="P")
                nc.scalar.activation(P[:], sc[:], AF.Exp, bias=shift_sb[:],
                                     scale=1.0)
                return P

            def transp(cl):
                # kT quadrants -> knat [g part, w cols] (+ preset ones)
                pkt = psum.tile([128, 512], f16, tag="pk", bufs=1)
                for gc in range(2):
                    for wc in range(2):
                        nc.tensor.matmul(
                            out=pkt[:, gc * 256 + wc * 128:gc * 256 + wc * 128 + 128],
                            lhsT=k_sl(cl, wc, 128, gc * 128),
                            rhs=ident_sb[:], is_transpose=True,
                            start=(wc == 0), stop=(wc == 1))
                kn = kn_ring[(g * G + cl) % 4]
                kn_out = bass.AP(kn[:].tensor, kn[:].offset,
                                 [kn[:].ap[0], [257, 2], [1, 256]])
                pk_in = bass.AP(pkt[:].tensor, pkt[:].offset,
                                [pkt[:].ap[0], [256, 2], [1, 256]])
                ke = CFG['knat'][cl % len(CFG['knat'])]
                if ke == 's':
                    nc.scalar.activation(kn_out, pk_in, AF.Copy)
                else:
                    nc.vector.tensor_copy(kn_out, pk_in)
                return kn

            def stage2(cl, P, kn, ob, xr, obslot):
                # out[h,w] (+ l in col 256) = sum_gc P^T block @ [knat|ones]
                po = psum.tile([128, 1024], f32, tag="po", bufs=2)
                for ht in range(2):
                    for gc in range(2):
                        nc.tensor.matmul(
                            out=po[:, ht * 512:ht * 512 + 257],
                            lhsT=P[:, gc * 256 + ht * 128:gc * 256 + ht * 128 + 128],
                            rhs=kn[:, gc * 257:gc * 257 + 257],
                            start=(gc == 0), stop=(gc == 1))
                rinv = stats.tile([128, 2], f32, tag="rinv")
                l_ap = bass.AP(po[:].tensor, po[:].offset + 256,
                               [po[:].ap[0], [512, 2]])
                nc.vector.reciprocal(rinv[:], l_ap)
                fe = CFG['final'][cl % len(CFG['final'])]
                if fe == 'v':
                    for ht in range(2):
                        nc.vector.scalar_tensor_tensor(
                            out=ob[:, obslot * 512 + ht * 256:obslot * 512 + ht * 256 + 256],
                            in0=po[:, ht * 512:ht * 512 + 256],
                            scalar=rinv[:, ht:ht + 1],
                            in1=xr[:, obslot * 512 + ht * 256:obslot * 512 + ht * 256 + 256],
                            op0=ALU.mult, op1=ALU.add)
                else:
                    tmp = tmppool.tile([128, 512], f16, tag="tmp")
                    for ht in range(2):
                        nc.scalar.activation(
                            tmp[:, ht * 256:ht * 256 + 256],
                            po[:, ht * 512:ht * 512 + 256],
                            AF.Copy, scale=rinv[:, ht:ht + 1])
                    nc.gpsimd.tensor_add(
                        ob[:, obslot * 512:obslot * 512 + 512], tmp[:],
                        xr[:, obslot * 512:obslot * 512 + 512])

            # CB-channel batches for xres-in and out DMAs; xres prefetched
            # one batch ahead. Software pipeline with D channels lookahead:
            # stage2(cl-D) runs after stage1/transp(cl), giving exp/knat a
            # full extra channel of slack before the out-mm consumes them.
            CB = CFG['chbatch']
            D = CFG['lookahead']

            def xr_fetch(b):
                xr = xrpool.tile([128, CB * 512], f16, tag="xr")
                nc.gpsimd.dma_start(
                    out=sb4(xr[:], [512, CB], [256, 2], [1, 256]),
                    in_=dram_ch(xres_t, g * G + b * CB, CB))
                return xr

            xrq = [xr0]
            pend = []
            obs = {}
            for cl in range(G + D):
                if cl == G - 32 and g + 1 < NG:
                    # prefetch next group's first two x chunks during Phase B
                    xt_pre = xt_fetch(0)
                if cl == G - 12 and g + 1 < NG:
                    xt_pre2 = xt_fetch(1)
                if cl < G:
                    if cl % CB == 0:
                        obs[cl // CB] = opool.tile([128, CB * 512], f16,
                                                   tag="ob", name="ob")
                        if cl // CB + 1 < G // CB:
                            xrq.append(xr_fetch(cl // CB + 1))
                    P = stage1(cl)
                    kn = transp(cl)
                    pend.append((cl, P, kn))
                if cl >= D:
                    c0, P0, kn0 = pend.pop(0)
                    b = c0 // CB
                    stage2(c0, P0, kn0, obs[b], xrq[0], c0 % CB)
                    if c0 % CB == CB - 1:
                        nc.gpsimd.dma_start(
                            out=dram_ch(out_t, g * G + b * CB, CB),
                            in_=sb4(obs[b][:], [512, CB], [256, 2], [1, 256]))
                        xrq.pop(0)
                        del obs[b]
    return nc


def _host_consts(W1, b1, W2, b2):
    delta = np.linalg.solve(
        np.eye(C, dtype=np.float64) + np.asarray(W2, np.float64),
        np.asarray(b2, np.float64))
    alpha_q = np.asarray(b1, np.float64) - np.asarray(W1, np.float64) @ delta
    wcat = np.empty((128, 2 * C), np.float16)
    for g in range(NG):
        for t, Wm in ((0, W1), (1, W2)):
            for cl in range(G):
                wcat[:, g * 128 + t * G + cl] = np.float16(Wm[g * G + cl, :])
    biasq = np.zeros((128, NG * 512), np.float16)
    for g in range(NG):
        for cl in range(G):
            biasq[:, g * 512 + cl * 8:g * 512 + cl * 8 + 8] = np.float16(
                alpha_q[g * G + cl])
    ident = np.eye(128, dtype=np.float16)
    return delta, {"wcat": wcat, "biasq": biasq, "ident": ident}


def _host_inputs(x_b, delta, consts):
    xres = (np.asarray(x_b, np.float64)
            + delta[:, None, None]).astype(np.float16)
    return {"xres": xres, **consts}


def kernel(x, W1, b1, W2, b2, _trace=False):
    import concourse.bass_utils as bass_utils

    nc = build_program(patch=True)
    _split_multi_waits(nc)

    delta, consts = _host_consts(W1, b1, W2, b2)
    in_maps = [_host_inputs(x[b], delta, consts) for b in range(B)]
    kw = {}
    if _trace:
        kw = dict(trace=True, trace_cores=[0])
    res = bass_utils.run_bass_kernel_spmd(
        nc, in_maps, core_ids=list(range(N_CORES)), **kw)
    out = np.stack([res.results[b]["out"] for b in range(B)], axis=0)
    if _trace:
        kernel._last_results = res
    return out.astype(np.float32)
